# revision 1
# baseline (speedup 1.0000x reference)
"""GAT GNN kernel for 8 Trainium2 NeuronCores (Bass, via PJRT/axon).

Strategy (per spec sharding_hint): partition dst nodes (and their incoming
edges) across 8 cores. Nodes are permuted by in-degree so each 128-node dst
tile has near-uniform degree -> tight ELL (padded CSR) slot grids. Per tile:
  - indirect-DMA gather of [prev | es | ed] rows for every edge slot
    (slot 0 = self loop, also supplies ed[dst] per partition)
  - on-chip segment softmax: z = es[src]+ed[dst]; lrelu; per-row (=per dst)
    max/exp/sum on DVE+ACT; p = exp(lz - m)
  - aggregation: feats *= p (DVE), reduce over slots (DVE)
  - out = (agg/den) @ W + b via PE (transpose + matmul), exploiting
    (sum_e a_e prev[src]) @ W == sum_e a_e (prev@W)[src]
Three launches (L1, L2, L3); host applies relu and computes next-layer
es/ed = prev @ (W@a) between launches, then mean/max-pools by graph and
applies the final linear. L2 and L3 share one compiled kernel.
"""
import os
import sys
import math

sys.path.insert(0, "/opt/trn_rl_repo")

import numpy as np

P = 128
F_OUT = 64
NEG_SLOPE = 0.2
N_CORES = 8
COLS_BUDGET = {130: 96, 66: 160}  # gather cols per group, by row width
MAX_NT = 4

_RUNNERS = {}


def _make_runner(nc, replicated_names):
    """jit the bass module over 8 cores via shard_map; returns fn(global_ins)->
    np [8*SHR, 64]. Inputs in replicated_names get PartitionSpec(None)."""
    import jax
    from jax.sharding import Mesh, PartitionSpec
    from jax.experimental.shard_map import shard_map
    import concourse.mybir as mybir
    from concourse.bass2jax import (_bass_exec_p, partition_id_tensor,
                                    install_neuronx_cc_hook)

    install_neuronx_cc_hook()
    nc.finalize()
    partition_name = nc.partition_id_tensor.name if nc.partition_id_tensor else None

    in_names, out_names, out_avals, zero_outs = [], [], [], []
    for alloc in nc.m.functions[0].allocations:
        if not isinstance(alloc, mybir.MemoryLocationSet):
            continue
        name = alloc.memorylocations[0].name
        if alloc.kind == "ExternalInput":
            if name != partition_name:
                in_names.append(name)
        elif alloc.kind == "ExternalOutput":
            shape = tuple(alloc.tensor_shape)
            dtype = mybir.dt.np(alloc.dtype)
            out_names.append(name)
            out_avals.append(jax.core.ShapedArray(shape, dtype))
            zero_outs.append(np.zeros(shape, dtype))
    n_params = len(in_names)
    all_in = in_names + out_names + ([partition_name] if partition_name else [])

    def _body(*args):
        operands = list(args)
        if partition_name is not None:
            operands.append(partition_id_tensor())
        return tuple(_bass_exec_p.bind(
            *operands,
            out_avals=tuple(out_avals), in_names=tuple(all_in),
            out_names=tuple(out_names), lowering_input_output_aliases=(),
            sim_require_finite=False, sim_require_nnan=False, nc=nc))

    devices = jax.devices()[:N_CORES]
    mesh = Mesh(np.asarray(devices), ("core",))
    in_specs = tuple(
        PartitionSpec(None) if n in replicated_names else PartitionSpec("core")
        for n in in_names) + (PartitionSpec("core"),) * len(out_names)
    out_specs = (PartitionSpec("core"),) * len(out_names)
    jfn = jax.jit(shard_map(_body, mesh=mesh, in_specs=in_specs,
                            out_specs=out_specs, check_rep=False),
                  keep_unused=True)

    def fn(global_ins):
        args = [global_ins[n] for n in in_names]
        args += [np.zeros((N_CORES * z.shape[0], *z.shape[1:]), z.dtype)
                 for z in zero_outs]
        outs = jfn(*args)
        jax.block_until_ready(outs)
        return np.asarray(outs[0])

    return fn, in_names


def _build_layer_kernel(RC, R_TOT, groups, totcols, shr_rows):
    """One GAT layer for one core's dst shard.

    RC: gathered row width (K_IN feats + es + ed). groups: list of
    (col_off, row_off, nt, Kg). Output: [shr_rows, 64] raw (no relu)."""
    import concourse.bacc as bacc
    import concourse.bass as bass
    import concourse.mybir as mybir
    import concourse.tile as tile
    from concourse.masks import make_identity

    DT = mybir.dt.float32
    A = mybir.AluOpType
    K_IN = RC - 2
    nc = bacc.Bacc("TRN2", target_bir_lowering=False, debug=False,
                   num_devices=N_CORES)
    tbl = nc.dram_tensor("tbl", [R_TOT, RC], DT, kind="ExternalInput")
    idx = nc.dram_tensor("idx", [P, totcols], mybir.dt.uint32,
                         kind="ExternalInput")
    W_d = nc.dram_tensor("w", [K_IN, F_OUT], DT, kind="ExternalInput")
    b_d = nc.dram_tensor("b", [P, F_OUT], DT, kind="ExternalInput")
    out_d = nc.dram_tensor("out", [shr_rows, F_OUT], DT, kind="ExternalOutput")

    with tile.TileContext(nc) as tc:
        with (tc.tile_pool(name="const", bufs=1) as cpool,
              tc.tile_pool(name="sb", bufs=2) as pool,
              tc.tile_pool(name="ps", bufs=2, space="PSUM") as pspool):
            ident = cpool.tile([P, P], DT)
            make_identity(nc, ident[:])
            w_sb = cpool.tile([K_IN, F_OUT], DT)
            nc.sync.dma_start(out=w_sb[:], in_=W_d[:])
            b_sb = cpool.tile([P, F_OUT], DT)
            nc.sync.dma_start(out=b_sb[:], in_=b_d[:])

            for (col_off, row_off, nt, Kg) in groups:
                cols = nt * Kg
                it = pool.tile([P, cols], mybir.dt.uint32, tag="idx")
                nc.sync.dma_start(out=it[:], in_=idx[:, col_off:col_off + cols])
                g = pool.tile([P, cols * RC], DT, tag="g")
                for cc in range(cols):
                    nc.gpsimd.indirect_dma_start(
                        out=g[:, cc * RC:(cc + 1) * RC], out_offset=None,
                        in_=tbl[:],
                        in_offset=bass.IndirectOffsetOnAxis(
                            ap=it[:, cc:cc + 1], axis=0))
                gb = g[:]
                pstep = gb.ap[0][0]

                def gap(off, dims):
                    return bass.AP(gb.tensor, gb.offset + off,
                                   [[pstep, P]] + dims)

                # z = es_slot + ed_own  (ed from self-loop slot 0 per tile)
                z = pool.tile([P, cols], DT, tag="z")
                nc.vector.tensor_tensor(
                    out=z[:],
                    in0=gap(K_IN, [[RC, cols]]),
                    in1=gap(K_IN + 1, [[Kg * RC, nt], [0, Kg]]),
                    op=A.add)
                # leaky relu (in place, exact): z = max(max(z, 0.2z), -30)
                zt = pool.tile([P, cols], DT, tag="zt")
                nc.vector.tensor_scalar_mul(zt[:], z[:], NEG_SLOPE)
                nc.vector.tensor_tensor(out=z[:], in0=z[:], in1=zt[:], op=A.max)
                nc.vector.tensor_scalar_max(z[:], z[:], -30.0)
                zv = z[:].rearrange("p (t k) -> p t k", k=Kg)
                nc.scalar.activation(z[:], z[:],
                                     mybir.ActivationFunctionType.Exp)
                # den and 1/den
                den = pool.tile([P, nt], DT, tag="den")
                nc.vector.tensor_reduce(out=den[:], in_=zv,
                                        axis=mybir.AxisListType.X, op=A.add)
                nc.vector.reciprocal(den[:], den[:])
                # feats *= p  (in place on gathered rows)
                zb = z[:]
                nc.vector.tensor_tensor(
                    out=gap(0, [[RC, cols], [1, K_IN]]),
                    in0=gap(0, [[RC, cols], [1, K_IN]]),
                    in1=bass.AP(zb.tensor, zb.offset,
                                [[zb.ap[0][0], P], [1, cols], [0, K_IN]]),
                    op=A.mult)
                # reduce over slots -> agg [P, nt*K_IN]
                agg = pool.tile([P, nt * K_IN], DT, tag="agg")
                nc.vector.tensor_reduce(
                    out=agg[:],
                    in_=gap(0, [[Kg * RC, nt], [1, K_IN], [RC, Kg]]),
                    axis=mybir.AxisListType.X, op=A.add)
                # agg *= 1/den
                db = den[:]
                nc.vector.tensor_tensor(
                    out=agg[:], in0=agg[:],
                    in1=bass.AP(db.tensor, db.offset,
                                [[db.ap[0][0], P], [1, nt], [0, K_IN]]),
                    op=A.mult)
                # transpose each tile's agg, then matmul with W
                psT = pspool.tile([K_IN, nt * P], DT, tag="psT")
                aggv = agg[:].rearrange("p (t f) -> p t f", f=K_IN)
                for t in range(nt):
                    nc.tensor.transpose(out=psT[:, t * P:(t + 1) * P],
                                        in_=aggv[:, t, :], identity=ident[:])
                aggT = pool.tile([K_IN, nt * P], DT, tag="aggT")
                nc.vector.tensor_copy(out=aggT[:], in_=psT[:])
                psO = pspool.tile([P, nt * F_OUT], DT, tag="psO")
                for t in range(nt):
                    nc.tensor.matmul(out=psO[:, t * F_OUT:(t + 1) * F_OUT],
                                     lhsT=aggT[:, t * P:(t + 1) * P],
                                     rhs=w_sb[:], start=True, stop=True)
                outt = pool.tile([P, nt * F_OUT], DT, tag="outt")
                bb = b_sb[:]
                nc.vector.tensor_tensor(
                    out=outt[:], in0=psO[:],
                    in1=bass.AP(bb.tensor, bb.offset,
                                [[bb.ap[0][0], P], [0, nt], [1, F_OUT]]),
                    op=A.add)
                # write rows: row (t, p) -> shard row row_off + t*128 + p
                ob = out_d[:]
                dst_ap = bass.AP(ob.tensor, ob.offset + row_off * F_OUT,
                                 [[F_OUT, P], [P * F_OUT, nt], [1, F_OUT]])
                nc.sync.dma_start(out=dst_ap, in_=outt[:])
    return nc


def _prep_graph(N, src, dst):
    """Degree-permuted ELL layout. Returns dict with ranks, tiles, groups,
    and per-core idx arrays."""
    deg = np.bincount(dst, minlength=N).astype(np.int64) + 1  # + self loop
    order = np.argsort(deg, kind="stable")     # node id per rank
    rank = np.empty(N, np.int64)
    rank[order] = np.arange(N)
    n_tiles = (N + P - 1) // P
    R_TOT = (n_tiles + 1) * P
    DUMMY = R_TOT - 1

    # CSR over dst ranks
    dstr = rank[dst]
    srcr = rank[src].astype(np.uint32)
    ord_e = np.argsort(dstr, kind="stable")
    dstr_s = dstr[ord_e]
    srcr_s = srcr[ord_e]
    indptr = np.searchsorted(dstr_s, np.arange(N + 1))

    # per-rank degree (incl self), padded ranks get 0 slots (all dummy)
    degr = np.zeros(R_TOT, np.int64)
    degr[:N] = deg[order]

    T_core = (n_tiles + N_CORES - 1) // N_CORES
    # K per tile position j (max over cores, tiles j*8+c), >=2
    K_hat = np.zeros(T_core, np.int64)
    for j in range(T_core):
        ts = [j * N_CORES + c for c in range(N_CORES) if j * N_CORES + c < n_tiles]
        K_hat[j] = max(2, max(int(degr[t * P:(t + 1) * P].max()) for t in ts))

    budget = None  # set by caller per RC
    return dict(order=order, rank=rank, n_tiles=n_tiles, R_TOT=R_TOT,
                DUMMY=DUMMY, srcr_s=srcr_s, indptr=indptr, degr=degr,
                T_core=T_core, K_hat=K_hat)


def _make_groups(K_hat, budget):
    groups = []
    j = 0
    T = len(K_hat)
    col_off = 0
    while j < T:
        nt = 1
        kg = int(K_hat[j])
        while (j + nt < T and nt < MAX_NT
               and (nt + 1) * max(kg, int(K_hat[j + nt])) <= budget):
            kg = max(kg, int(K_hat[j + nt]))
            nt += 1
        groups.append((col_off, j * P, nt, kg))
        col_off += nt * kg
        j += nt
    return groups, col_off


def _fill_idx(gp, groups, totcols):
    """Per-core idx arrays [P, totcols] uint32 (slot 0 = self rank)."""
    srcr_s, indptr, degr = gp["srcr_s"], gp["indptr"], gp["degr"]
    n_tiles, DUMMY, T_core = gp["n_tiles"], gp["DUMMY"], gp["T_core"]
    N = len(indptr) - 1
    idxs = np.full((N_CORES, P, totcols), DUMMY, np.uint32)
    for c in range(N_CORES):
        for (col_off, row_off, nt, Kg) in groups:
            for t in range(nt):
                j = row_off // P + t
                tile_id = j * N_CORES + c
                if tile_id >= n_tiles:
                    continue
                r0 = tile_id * P
                ranks = np.arange(r0, r0 + P)
                real = ranks < N
                co = col_off + t * Kg
                # self loop slot
                idxs[c, :, co][real] = ranks[real].astype(np.uint32)
                # edge slots
                lo = indptr[np.minimum(ranks, N - 1)]
                hi = indptr[np.minimum(ranks, N - 1) + 1]
                L = np.where(real, hi - lo, 0)
                kmax = int(L.max()) if L.size else 0
                for k in range(kmax):
                    sel = k < L
                    idxs[c, sel, co + 1 + k] = srcr_s[lo[sel] + k]
    return idxs


def kernel(x, edge_index, batch, W1, as1, ad1, b1, W2, as2, ad2, b2,
           W3, as3, ad3, b3, linW, linb):
    import jax

    x = np.asarray(x, np.float32)
    edge_index = np.asarray(edge_index)
    batch = np.asarray(batch)
    W1, W2, W3 = (np.asarray(w, np.float32) for w in (W1, W2, W3))
    as1, ad1, as2, ad2, as3, ad3 = (np.asarray(a, np.float32)
                                    for a in (as1, ad1, as2, ad2, as3, ad3))
    b1, b2, b3 = (np.asarray(b, np.float32) for b in (b1, b2, b3))
    linW = np.asarray(linW, np.float32)
    linb = np.asarray(linb, np.float32)

    N, F_in = x.shape
    src = edge_index[0]
    dst = edge_index[1]

    gp = _prep_graph(N, src, dst)
    R_TOT, order, rank = gp["R_TOT"], gp["order"], gp["rank"]
    T_core = gp["T_core"]
    shr = T_core * P

    key = (N, int(edge_index.shape[1]))
    if key not in _RUNNERS:
        g1, tc1 = _make_groups(gp["K_hat"], COLS_BUDGET[F_in + 2])
        g2, tc2 = _make_groups(gp["K_hat"], COLS_BUDGET[F_OUT + 2])
        idx1 = _fill_idx(gp, g1, tc1)
        idx2 = _fill_idx(gp, g2, tc2)
        nc1 = _build_layer_kernel(F_in + 2, R_TOT, g1, tc1, shr)
        fn1, _ = _make_runner(nc1, {"tbl", "w", "b"})
        nc2 = _build_layer_kernel(F_OUT + 2, R_TOT, g2, tc2, shr)
        fn2, _ = _make_runner(nc2, {"tbl", "w", "b"})
        from jax.sharding import Mesh, PartitionSpec, NamedSharding
        mesh = Mesh(np.asarray(jax.devices()[:N_CORES]), ("core",))
        sh = NamedSharding(mesh, PartitionSpec("core"))
        idx1g = jax.device_put(idx1.reshape(N_CORES * P, tc1), sh)
        idx2g = jax.device_put(idx2.reshape(N_CORES * P, tc2), sh)
        _RUNNERS[key] = (fn1, fn2, idx1g, idx2g)
    fn1, fn2, idx1g, idx2g = _RUNNERS[key]

    def build_table(prev, wa, wd, RC):
        """prev [N, K] by node -> table [R_TOT, RC] by rank."""
        K = prev.shape[1]
        t = np.zeros((R_TOT, RC), np.float32)
        t[rank[np.arange(N)], :K] = prev
        es = prev.astype(np.float64) @ wa.astype(np.float64)
        ed = prev.astype(np.float64) @ wd.astype(np.float64)
        t[rank[np.arange(N)], K] = es.astype(np.float32)
        t[rank[np.arange(N)], K + 1] = ed.astype(np.float32)
        t[gp["DUMMY"], K] = -200.0  # padding slots contribute exp(-inf)=0
        return t

    def unshard(o):
        """[8*shr, 64] -> by-node [N, 64]."""
        o = o.reshape(N_CORES, T_core, P, F_OUT)
        full = np.zeros((gp["n_tiles"] * P, F_OUT), np.float32)
        for c in range(N_CORES):
            for j in range(T_core):
                tile_id = j * N_CORES + c
                if tile_id < gp["n_tiles"]:
                    full[tile_id * P:(tile_id + 1) * P] = o[c, j]
        return full[rank[np.arange(N)]]

    import time
    times = []

    def run(fn, table, idxg, W, b):
        ins = {"tbl": table, "idx": idxg,
               "w": np.ascontiguousarray(W),
               "b": np.tile(b.reshape(1, F_OUT), (P, 1))}
        t0 = time.perf_counter()
        o = fn(ins)
        times.append(time.perf_counter() - t0)
        return unshard(o)

    t1 = build_table(x, W1 @ as1, W1 @ ad1, F_in + 2)
    out1 = run(fn1, t1, idx1g, W1, b1)
    prev2 = np.maximum(out1, 0.0)
    t2 = build_table(prev2, W2 @ as2, W2 @ ad2, F_OUT + 2)
    out2 = run(fn2, t2, idx2g, W2, b2)
    prev3 = np.maximum(out2, 0.0)
    t3 = build_table(prev3, W3 @ as3, W3 @ ad3, F_OUT + 2)
    h = run(fn2, t3, idx2g, W3, b3)

    kernel._launch_times = times

    # global mean+max pool by graph (batch sorted), then final linear
    G = 512
    b64 = np.asarray(batch).astype(np.int64)
    starts = np.searchsorted(b64, np.arange(G))
    ends = np.searchsorted(b64, np.arange(G), side="right")
    counts = (ends - starts).astype(np.float32)
    gmean = np.zeros((G, F_OUT), np.float32)
    gmax = np.zeros((G, F_OUT), np.float32)
    ne = counts > 0
    if ne.any():
        sums = np.add.reduceat(h, starts[ne], axis=0)
        gmean[ne] = sums / counts[ne, None]
        gmax[ne] = np.array([h[starts[g]:ends[g]].max(0)
                             for g in np.flatnonzero(ne)], np.float32)
    pooled = np.concatenate([gmean, gmax], axis=1)
    return (pooled @ linW + linb).astype(np.float32)



# revision 7
# speedup vs baseline: 1174.5840x; 1174.5840x over previous
"""GAT GNN kernel for 8 Trainium2 NeuronCores (Bass, via PJRT/axon).

Fully fused single-launch design. Partition dst nodes (and their incoming
edges) across 8 cores; nodes permuted by in-degree so each 128-node dst tile
has near-uniform degree -> tight ELL slot grids. All three GAT layers plus the
graph mean/max pooling run in ONE device program; the per-layer gather tables
live in device DRAM and are rebuilt between layers with on-device AllGather
collectives (no host round trips):

  stage0: per-core h1|es1|ed1 = xg @ [W1 | W1 as1 | W1 ad1]  -> AllGather tbl1
  layer l: ELL gather of [h | es | ed] rows per edge slot; on-chip segment
    softmax (lrelu, exp, rowsum); weighted reduce -> out = agg/den + b;
    relu (l<3); next contribution = out @ [W' | W'as' | W'ad'] -> AllGather
  pooling: per-core 64 graphs; ELL gather of member rows from the gathered
    h3 table (zero pads for sum, -1e30 pads for max); segment mean+max ->
    out [64, 128] per core ([512, 128] global). Host: final linear.

Host keeps device-resident caches (content-hashed) for x, the edge/pool idx
grids and weights, so warm launches transfer nothing big.
"""
import sys
import zlib

sys.path.insert(0, "/opt/trn_rl_repo")

import numpy as np

P = 128
H = 64
RC = 66                 # h(64) | es | ed
N_CORES = 8
NEG_SLOPE = 0.2
NG = 512
COLS_BUDGET = 160       # gather cols per group (RC=66 -> 42KB/partition tile)
MAX_NT = 4
POOL_CHUNK = 128

_CACHE = {}


def _make_runner(nc, replicated_names):
    """jit the bass module over 8 cores via shard_map; returns fn(ins)->np.
    Inputs in replicated_names get PartitionSpec(None)."""
    import jax
    from jax.sharding import Mesh, PartitionSpec
    from jax.experimental.shard_map import shard_map
    import concourse.mybir as mybir
    from concourse.bass2jax import (_bass_exec_p, partition_id_tensor,
                                    install_neuronx_cc_hook)

    install_neuronx_cc_hook()
    nc.finalize()
    partition_name = nc.partition_id_tensor.name if nc.partition_id_tensor else None

    in_names, out_names, out_avals, zero_outs = [], [], [], []
    for alloc in nc.m.functions[0].allocations:
        if not isinstance(alloc, mybir.MemoryLocationSet):
            continue
        name = alloc.memorylocations[0].name
        if alloc.kind == "ExternalInput":
            if name != partition_name:
                in_names.append(name)
        elif alloc.kind == "ExternalOutput":
            shape = tuple(alloc.tensor_shape)
            dtype = mybir.dt.np(alloc.dtype)
            out_names.append(name)
            out_avals.append(jax.core.ShapedArray(shape, dtype))
            zero_outs.append(np.zeros(shape, dtype))
    all_in = in_names + out_names + ([partition_name] if partition_name else [])

    def _body(*args):
        operands = list(args)
        if partition_name is not None:
            operands.append(partition_id_tensor())
        return tuple(_bass_exec_p.bind(
            *operands,
            out_avals=tuple(out_avals), in_names=tuple(all_in),
            out_names=tuple(out_names), lowering_input_output_aliases=(),
            sim_require_finite=False, sim_require_nnan=False, nc=nc))

    devices = jax.devices()[:N_CORES]
    mesh = Mesh(np.asarray(devices), ("core",))
    in_specs = tuple(
        PartitionSpec(None) if n in replicated_names else PartitionSpec("core")
        for n in in_names) + (PartitionSpec("core"),) * len(out_names)
    out_specs = (PartitionSpec("core"),) * len(out_names)
    jfn = jax.jit(shard_map(_body, mesh=mesh, in_specs=in_specs,
                            out_specs=out_specs, check_rep=False),
                  keep_unused=True)

    def fn(global_ins):
        args = [global_ins[n] for n in in_names]
        args += [np.zeros((N_CORES * z.shape[0], *z.shape[1:]), z.dtype)
                 for z in zero_outs]
        outs = jfn(*args)
        jax.block_until_ready(outs)
        return np.asarray(outs[0])

    return fn, in_names, mesh


def _make_groups(K_hat, budget):
    groups = []
    j = 0
    T = len(K_hat)
    col_off = 0
    while j < T:
        nt = 1
        kg = int(K_hat[j])
        while (j + nt < T and nt < MAX_NT
               and (nt + 1) * max(kg, int(K_hat[j + nt])) <= budget):
            kg = max(kg, int(K_hat[j + nt]))
            nt += 1
        groups.append((col_off, j * P, nt, kg))
        col_off += nt * kg
        j += nt
    return groups, col_off


def _prep_graph(N, src, dst, batch):
    """All data-dependent layout: ELL idx grids (gathered-row ids), groups,
    pooling idx, xg permutation."""
    deg = np.bincount(dst, minlength=N).astype(np.int64) + 1  # + self loop
    order = np.argsort(deg, kind="stable")
    rank = np.empty(N, np.int64)
    rank[order] = np.arange(N)
    n_tiles = (N + P - 1) // P
    T_core = (n_tiles + N_CORES - 1) // N_CORES
    SHR = T_core * P
    SH = SHR + 2          # + dummy rows A (zeros, es=-200) and B (-1e30)
    TOT = N_CORES * SH
    DUMA = SHR            # global gathered-row id of shard0 dummy A
    DUMB = SHR + 1

    # rank r -> gathered table row / xg row
    RTOT = n_tiles * P
    rr = np.arange(RTOT)
    tt = rr // P
    pp = rr % P
    grow = (tt % N_CORES) * SH + (tt // N_CORES) * P + pp
    xgrow = (tt % N_CORES) * SHR + (tt // N_CORES) * P + pp

    # degrees by rank -> per-tile-position K (max over the 8 cores' tiles)
    degr = np.zeros(RTOT, np.int64)
    degr[:N] = deg[order]
    deg_tile = degr.reshape(n_tiles, P).max(1)
    K_hat = np.zeros(T_core, np.int64)
    for j in range(T_core):
        hi = min((j + 1) * N_CORES, n_tiles)
        K_hat[j] = max(2, int(deg_tile[j * N_CORES:hi].max()))
    groups, TOTCOLS = _make_groups(K_hat, COLS_BUDGET)

    colstart = np.zeros(T_core, np.int64)
    for (col_off, row_off, nt, Kg) in groups:
        j0 = row_off // P
        for t in range(nt):
            colstart[j0 + t] = col_off + t * Kg

    # ELL idx grids: slot 0 = self, then edges sorted by dst rank
    idxs = np.full((N_CORES, P, TOTCOLS), DUMA, np.uint32)
    r_all = rank  # rank of node n
    t_r = r_all // P
    idxs[t_r % N_CORES, r_all % P, colstart[t_r // N_CORES]] = \
        grow[r_all].astype(np.uint32)
    dstr = rank[dst]
    ord_e = np.argsort(dstr, kind="stable")
    dstr_s = dstr[ord_e]
    grow_src = grow[rank[src[ord_e]]].astype(np.uint32)
    indptr = np.searchsorted(dstr_s, np.arange(RTOT + 1))
    k_e = np.arange(len(dstr_s)) - indptr[dstr_s]
    t_e = dstr_s // P
    idxs[t_e % N_CORES, dstr_s % P,
         colstart[t_e // N_CORES] + 1 + k_e] = grow_src

    # pooling: batch is sorted; graph g -> core g//64, slot g%64
    b64 = batch.astype(np.int64)
    starts = np.searchsorted(b64, np.arange(NG))
    ends = np.searchsorted(b64, np.arange(NG), side="right")
    counts = (ends - starts).astype(np.int64)
    maxc = max(1, int(counts.max()))
    nchunks = (maxc + POOL_CHUNK - 1) // POOL_CHUNK
    KP = nchunks * POOL_CHUNK
    idxp = np.empty((N_CORES, 64, 2 * KP), np.uint32)
    idxp[:, :, :KP] = DUMA
    idxp[:, :, KP:] = DUMB
    n_all = np.arange(N)
    g_n = b64
    k_n = n_all - starts[g_n]
    gval = grow[rank].astype(np.uint32)
    idxp[g_n // 64, g_n % 64, k_n] = gval
    idxp[g_n // 64, g_n % 64, KP + k_n] = gval
    invc = np.zeros((NG,), np.float32)
    nz = counts > 0
    invc[nz] = 1.0 / counts[nz]
    invc = invc.reshape(N_CORES, 64, 1)

    # xg permutation: gathered xg row -> node id (N = zero pad row)
    pg = np.full(N_CORES * SHR, N, np.int64)
    pg[xgrow[:N]] = order

    slots = sum(P * nt * Kg for (_, _, nt, Kg) in groups) * N_CORES
    return dict(order=order, rank=rank, n_tiles=n_tiles, T_core=T_core,
                SHR=SHR, SH=SH, TOT=TOT, groups=groups, TOTCOLS=TOTCOLS,
                idxs=idxs, idxp=idxp, invc=invc, counts=counts, KP=KP,
                nchunks=nchunks, pg=pg, slots=slots)


def _build_kernel(T_core, SH, TOT, groups, TOTCOLS, KP, nchunks):
    """One fused program: stage0 + 3 GAT layers + AllGathers + pooling."""
    import concourse.bacc as bacc
    import concourse.bass as bass
    import concourse.mybir as mybir
    import concourse.tile as tile
    from concourse.masks import make_identity

    DT = mybir.dt.float32
    U32 = mybir.dt.uint32
    A = mybir.AluOpType
    SHR = T_core * P
    RG = [list(range(N_CORES))]

    nc = bacc.Bacc("TRN2", target_bir_lowering=False, debug=False,
                   num_devices=N_CORES)
    xg_d = nc.dram_tensor("xg", [SHR, 128], DT, kind="ExternalInput")
    idx_d = nc.dram_tensor("idx", [P, TOTCOLS], U32, kind="ExternalInput")
    idxp_d = nc.dram_tensor("idxp", [64, 2 * KP], U32, kind="ExternalInput")
    invc_d = nc.dram_tensor("invc", [64, 1], DT, kind="ExternalInput")
    w1_d = nc.dram_tensor("w1", [128, RC], DT, kind="ExternalInput")
    w2_d = nc.dram_tensor("w2", [64, RC], DT, kind="ExternalInput")
    w3_d = nc.dram_tensor("w3", [64, RC], DT, kind="ExternalInput")
    bias_d = nc.dram_tensor("bias", [P, 3 * 64], DT, kind="ExternalInput")
    dmy_d = nc.dram_tensor("dmy", [2, RC], DT, kind="ExternalInput")
    out_d = nc.dram_tensor("out", [64, 128], DT, kind="ExternalOutput")

    with tile.TileContext(nc) as tc:
        with (tc.tile_pool(name="const", bufs=1) as cpool,
              tc.tile_pool(name="sb", bufs=2) as pool,
              tc.tile_pool(name="ps", bufs=2, space="PSUM") as pspool,
              tc.tile_pool(name="dram", bufs=1, space="DRAM") as dpool):
            ident = cpool.tile([P, P], DT)
            make_identity(nc, ident[:])
            w1s = cpool.tile([128, RC], DT)
            nc.sync.dma_start(out=w1s[:], in_=w1_d[:])
            w2s = cpool.tile([64, RC], DT)
            nc.sync.dma_start(out=w2s[:], in_=w2_d[:])
            w3s = cpool.tile([64, RC], DT)
            nc.sync.dma_start(out=w3s[:], in_=w3_d[:])
            bias_s = cpool.tile([P, 3 * 64], DT)
            nc.sync.dma_start(out=bias_s[:], in_=bias_d[:])
            dmy_s = cpool.tile([2, RC], DT)
            nc.sync.dma_start(out=dmy_s[:], in_=dmy_d[:])
            ivc = cpool.tile([64, 1], DT)
            nc.sync.dma_start(out=ivc[:], in_=invc_d[:])
            ip = cpool.tile([64, 2 * KP], U32)
            nc.sync.dma_start(out=ip[:], in_=idxp_d[:])

            c1 = dpool.tile([SH, RC], DT, name="c1")
            c2 = dpool.tile([SH, RC], DT, name="c2")
            c3 = dpool.tile([SH, RC], DT, name="c3")
            hc = dpool.tile([SH, 64], DT, name="hc")
            t1 = dpool.tile([TOT, RC], DT, name="t1", addr_space="Shared")
            t2 = dpool.tile([TOT, RC], DT, name="t2", addr_space="Shared")
            t3 = dpool.tile([TOT, RC], DT, name="t3", addr_space="Shared")
            hall = dpool.tile([TOT, 64], DT, name="hall", addr_space="Shared")

            for c in (c1, c2, c3):
                nc.sync.dma_start(out=c[SHR:SH, :], in_=dmy_s[:])
            nc.sync.dma_start(out=hc[SHR:SH, :], in_=dmy_s[:, 0:64])

            # ---- stage0: contribution1 = xg @ Wext1 ----
            for k in range(T_core):
                xt = pool.tile([P, 128], DT, tag="xt")
                nc.sync.dma_start(out=xt[:], in_=xg_d[k * P:(k + 1) * P, :])
                psX = pspool.tile([128, P], DT, tag="psX")
                nc.tensor.transpose(out=psX[:], in_=xt[:], identity=ident[:])
                xT = pool.tile([128, P], DT, tag="xT")
                nc.vector.tensor_copy(out=xT[:], in_=psX[:])
                psC = pspool.tile([P, RC], DT, tag="psC")
                nc.tensor.matmul(out=psC[:], lhsT=xT[:], rhs=w1s[:],
                                 start=True, stop=True)
                ct = pool.tile([P, RC], DT, tag="ct")
                nc.vector.tensor_copy(out=ct[:], in_=psC[:])
                nc.sync.dma_start(out=c1[k * P:(k + 1) * P, :], in_=ct[:])
            nc.gpsimd.collective_compute(
                "AllGather", A.bypass, replica_groups=RG,
                ins=[c1.opt()], outs=[t1.opt()])

            # ---- GAT layers ----
            plan = [(t1, w2s, c2, t2), (t2, w3s, c3, t3), (t3, None, hc, hall)]
            for li, (tbl, wn, cnx, tnx) in enumerate(plan):
                boff = li * 64
                for (col_off, row_off, nt, Kg) in groups:
                    cols = nt * Kg
                    it = pool.tile([P, cols], U32, tag="it")
                    nc.sync.dma_start(out=it[:],
                                      in_=idx_d[:, col_off:col_off + cols])
                    g = pool.tile([P, cols * RC], DT, tag="g")
                    # HW indirect DMA takes ONE index per partition and
                    # streams the dest extent contiguously from it -> one
                    # instruction per slot column.
                    for cc in range(cols):
                        nc.gpsimd.indirect_dma_start(
                            out=g[:, cc * RC:(cc + 1) * RC], out_offset=None,
                            in_=tbl[:],
                            in_offset=bass.IndirectOffsetOnAxis(
                                ap=it[:, cc:cc + 1], axis=0))
                    gb = g[:]
                    pstep = gb.ap[0][0]

                    def gap(off, dims):
                        return bass.AP(gb.tensor, gb.offset + off,
                                       [[pstep, P]] + dims)

                    # z = es[src] + ed[dst] (ed from self slot 0 per tile)
                    z = pool.tile([P, cols], DT, tag="z")
                    nc.vector.tensor_tensor(
                        out=z[:], in0=gap(64, [[RC, cols]]),
                        in1=gap(65, [[Kg * RC, nt], [0, Kg]]), op=A.add)
                    zt = pool.tile([P, cols], DT, tag="zt")
                    nc.vector.tensor_scalar_mul(zt[:], z[:], NEG_SLOPE)
                    nc.vector.tensor_tensor(out=z[:], in0=z[:], in1=zt[:],
                                            op=A.max)
                    nc.vector.tensor_scalar_max(z[:], z[:], -30.0)
                    nc.scalar.activation(z[:], z[:],
                                         mybir.ActivationFunctionType.Exp)
                    den = pool.tile([P, nt], DT, tag="den")
                    nc.vector.tensor_reduce(
                        out=den[:], in_=z[:].rearrange("p (t k) -> p t k", k=Kg),
                        axis=mybir.AxisListType.X, op=A.add)
                    nc.vector.reciprocal(den[:], den[:])
                    zb = z[:]
                    nc.vector.tensor_tensor(
                        out=gap(0, [[RC, cols], [1, 64]]),
                        in0=gap(0, [[RC, cols], [1, 64]]),
                        in1=bass.AP(zb.tensor, zb.offset,
                                    [[zb.ap[0][0], P], [1, cols], [0, 64]]),
                        op=A.mult)
                    agg = pool.tile([P, nt * 64], DT, tag="agg")
                    nc.vector.tensor_reduce(
                        out=agg[:],
                        in_=gap(0, [[Kg * RC, nt], [1, 64], [RC, Kg]]),
                        axis=mybir.AxisListType.X, op=A.add)
                    db = den[:]
                    nc.vector.tensor_tensor(
                        out=agg[:], in0=agg[:],
                        in1=bass.AP(db.tensor, db.offset,
                                    [[db.ap[0][0], P], [1, nt], [0, 64]]),
                        op=A.mult)
                    bb = bias_s[:]
                    outt = pool.tile([P, nt * 64], DT, tag="outt")
                    nc.vector.tensor_tensor(
                        out=outt[:], in0=agg[:],
                        in1=bass.AP(bb.tensor, bb.offset + boff,
                                    [[bb.ap[0][0], P], [0, nt], [1, 64]]),
                        op=A.add)
                    if li < 2:
                        nc.vector.tensor_scalar_max(outt[:], outt[:], 0.0)
                        for t in range(nt):
                            psT = pspool.tile([64, P], DT, tag="psT")
                            nc.tensor.transpose(
                                out=psT[:], in_=outt[:, t * 64:(t + 1) * 64],
                                identity=ident[:])
                            aT = pool.tile([64, P], DT, tag="aT")
                            nc.vector.tensor_copy(out=aT[:], in_=psT[:])
                            psN = pspool.tile([P, RC], DT, tag="psN")
                            nc.tensor.matmul(out=psN[:], lhsT=aT[:], rhs=wn[:],
                                             start=True, stop=True)
                            cn = pool.tile([P, RC], DT, tag="cn")
                            nc.vector.tensor_copy(out=cn[:], in_=psN[:])
                            nc.sync.dma_start(
                                out=cnx[row_off + t * P:row_off + (t + 1) * P, :],
                                in_=cn[:])
                    else:
                        for t in range(nt):
                            nc.sync.dma_start(
                                out=cnx[row_off + t * P:row_off + (t + 1) * P, :],
                                in_=outt[:, t * 64:(t + 1) * 64])
                nc.gpsimd.collective_compute(
                    "AllGather", A.bypass, replica_groups=RG,
                    ins=[cnx.opt()], outs=[tnx.opt()])

            # ---- pooling: per-core 64 graphs, mean + max over members ----
            accs = cpool.tile([64, 64], DT)
            accm = cpool.tile([64, 64], DT)
            for ch in range(nchunks):
                for which in range(2):  # 0 = sum (zero pads), 1 = max (-1e30)
                    gp = pool.tile([64, POOL_CHUNK * 64], DT, tag="gp")
                    o = which * KP + ch * POOL_CHUNK
                    for k in range(POOL_CHUNK):
                        nc.gpsimd.indirect_dma_start(
                            out=gp[:, k * 64:(k + 1) * 64], out_offset=None,
                            in_=hall[:],
                            in_offset=bass.IndirectOffsetOnAxis(
                                ap=ip[:, o + k:o + k + 1], axis=0))
                    gpb = gp[:]
                    part = pool.tile([64, 64], DT, tag="part")
                    nc.vector.tensor_reduce(
                        out=part[:],
                        in_=bass.AP(gpb.tensor, gpb.offset,
                                    [[gpb.ap[0][0], 64], [1, 64],
                                     [64, POOL_CHUNK]]),
                        axis=mybir.AxisListType.X,
                        op=A.add if which == 0 else A.max)
                    acc = accs if which == 0 else accm
                    if ch == 0:
                        nc.vector.tensor_copy(out=acc[:], in_=part[:])
                    else:
                        nc.vector.tensor_tensor(
                            out=acc[:], in0=acc[:], in1=part[:],
                            op=A.add if which == 0 else A.max)
            po = pool.tile([64, 128], DT, tag="po")
            iv = ivc[:]
            nc.vector.tensor_tensor(
                out=po[:, 0:64], in0=accs[:],
                in1=bass.AP(iv.tensor, iv.offset, [[iv.ap[0][0], 64], [0, 64]]),
                op=A.mult)
            nc.vector.tensor_copy(out=po[:, 64:128], in_=accm[:])
            nc.sync.dma_start(out=out_d[:], in_=po[:])
    return nc


def _wext(W, a_s, a_d):
    W64 = W.astype(np.float64)
    es = (W64 @ a_s.astype(np.float64)).astype(np.float32)
    ed = (W64 @ a_d.astype(np.float64)).astype(np.float32)
    return np.ascontiguousarray(
        np.concatenate([W.astype(np.float32), es[:, None], ed[:, None]], 1))


def kernel(x, edge_index, batch, W1, as1, ad1, b1, W2, as2, ad2, b2,
           W3, as3, ad3, b3, linW, linb):
    import jax
    from jax.sharding import NamedSharding, PartitionSpec
    import time

    x = np.ascontiguousarray(np.asarray(x, np.float32))
    ei = np.ascontiguousarray(np.asarray(edge_index))
    bt = np.ascontiguousarray(np.asarray(batch))
    Ws = [np.asarray(w, np.float32) for w in (W1, W2, W3)]
    avs = [np.asarray(a, np.float32)
           for a in (as1, ad1, as2, ad2, as3, ad3)]
    bs = [np.asarray(b, np.float32) for b in (b1, b2, b3)]
    linW = np.asarray(linW, np.float32)
    linb = np.asarray(linb, np.float32)

    N = x.shape[0]
    gkey = (N, ei.shape[1], zlib.crc32(ei.tobytes()), zlib.crc32(bt.tobytes()))
    if gkey not in _CACHE:
        gp = _prep_graph(N, ei[0].astype(np.int64), ei[1].astype(np.int64),
                         bt)
        nc = _build_kernel(gp["T_core"], gp["SH"], gp["TOT"], gp["groups"],
                           gp["TOTCOLS"], gp["KP"], gp["nchunks"])
        fn, in_names, mesh = _make_runner(
            nc, {"w1", "w2", "w3", "bias", "dmy"})
        shard = NamedSharding(mesh, PartitionSpec("core"))
        repl = NamedSharding(mesh, PartitionSpec())
        dev = {
            "idx": jax.device_put(
                gp["idxs"].reshape(N_CORES * P, gp["TOTCOLS"]), shard),
            "idxp": jax.device_put(
                gp["idxp"].reshape(N_CORES * 64, 2 * gp["KP"]), shard),
            "invc": jax.device_put(
                gp["invc"].reshape(N_CORES * 64, 1), shard),
        }
        _CACHE[gkey] = dict(gp=gp, nc=nc, fn=fn, shard=shard, repl=repl,
                            dev=dev, xkey=None, wkey=None)
    st = _CACHE[gkey]
    gp = st["gp"]

    xkey = zlib.crc32(x.tobytes())
    if st["xkey"] != xkey:
        xp = np.concatenate([x, np.zeros((1, x.shape[1]), np.float32)], 0)
        xg = np.ascontiguousarray(xp[gp["pg"]])
        st["dev"]["xg"] = jax.device_put(xg, st["shard"])
        st["xkey"] = xkey
        st["xg_np"] = xg

    wbytes = b"".join(a.tobytes() for a in Ws + avs + bs)
    wkey = zlib.crc32(wbytes)
    if st["wkey"] != wkey:
        wx = [_wext(Ws[0], avs[0], avs[1]),
              _wext(Ws[1], avs[2], avs[3]),
              _wext(Ws[2], avs[4], avs[5])]
        bias_t = np.tile(np.concatenate(bs).reshape(1, 3 * 64), (P, 1))
        dmy = np.zeros((2, RC), np.float32)
        dmy[0, 64] = -200.0
        dmy[1, :] = -1e30
        for n, a in (("w1", wx[0]), ("w2", wx[1]), ("w3", wx[2]),
                     ("bias", bias_t), ("dmy", dmy)):
            st["dev"][n] = jax.device_put(np.ascontiguousarray(a), st["repl"])
        st["wkey"] = wkey

    t0 = time.perf_counter()
    pooled = st["fn"](st["dev"])
    dt = time.perf_counter() - t0
    kernel._launch_times = [dt]

    counts = gp["counts"]
    pooled = np.array(pooled)
    pooled[counts == 0] = 0.0

    # expose for test.py profiling
    kernel._nc = st["nc"]
    SHR = gp["SHR"]
    kernel._in_maps = [
        {"xg": st["xg_np"][c * SHR:(c + 1) * SHR],
         "idx": gp["idxs"][c],
         "idxp": gp["idxp"][c],
         "invc": gp["invc"][c],
         "w1": np.asarray(st["dev"]["w1"]),
         "w2": np.asarray(st["dev"]["w2"]),
         "w3": np.asarray(st["dev"]["w3"]),
         "bias": np.asarray(st["dev"]["bias"]),
         "dmy": np.asarray(st["dev"]["dmy"])}
        for c in range(N_CORES)]
    kernel._stats = dict(slots=gp["slots"], TOTCOLS=gp["TOTCOLS"],
                         KP=gp["KP"], groups=len(gp["groups"]))

    return (pooled @ linW + linb).astype(np.float32)
